# revision 49
# baseline (speedup 1.0000x reference)
"""MoE routed-expert kernel for Trainium2 (8 NeuronCores, SPMD).

Problem: N=16384 tokens, D=768, H=768, C=2, E=20 experts.
  y[n] = relu(x[n] @ W1[e] + b1[e]) @ W2[e] + b2[e],  e = component_idx[n]

Strategy
--------
Host (numpy): sort tokens by expert; split the largest expert groups in
half until there are 24 pieces; deal 8 pieces per slot group (3 slots x 8
cores); caps = per-group max (provably minimal sum-of-maxima).  Every core
runs the SAME static program; which expert a slot holds is just which
weights/tokens the host stages into that core's buffers.

Device (Bass/Tile, per core):
- Layer 1: 6x6 accumulating fp16 matmuls per chunk (chunks of 256..464
  tokens), relu fused on ScalarE.  Layer 2: 4-way PE column groups (32-wide
  tile_position, truly concurrent on HW) + a selector matmul.
- Software pipelining: the PE stream is [L1(k)] [L2grp(k-1)] [sel(k-2)] so
  the in-order PE queue never waits on relu/copy chains.
- DMA: every queued DMA costs ~0.7us serial on the sync HWDGE queue
  regardless of size, so transfers are consolidated into per-slot "blob"
  DMAs (w1+x+w2 in one shot for slots 1-2; w1_d0+x fused for the first
  chunk so a single semaphore gates the first matmul).  b1/b2 are all-zero
  in this workload -> no DMA at all (fallback path if nonzero).
"""

import math

import numpy as np

import concourse.bass as bass  # noqa: F401
import concourse.mybir as mybir
from concourse import bacc
from concourse.bass_utils import run_bass_kernel_spmd
from concourse.tile import TileContext

F32 = mybir.dt.float32
F16 = mybir.dt.float16
MM_DT = F16
MM_NP = np.float16

N_CORES = 8
N_SLOTS = 3
D = 768
H = 768
C = 2
DT = D // 128  # 6 d-tiles
HT = H // 128  # 6 h-tiles
MAX_CHUNK = 512  # one PSUM bank holds 512 fp32 -> matmul free dim cap
FIRST_CHUNK = 192  # small first chunk -> early PE start

L2_PACK = True
L2_M = 32  # pad W2's C=2 cols to a full 32-wide column group
W2M = L2_M if L2_PACK else C


def _chunk_list(cap: int, slot: int) -> list[int]:
    """Chunk sizes for a slot's cap. Slot 0 leads with a small chunk for
    an early PE start; the last slot ends with a 128 tail so only a short
    relu->L2->add->DMA chain trails the final big matmul."""
    if slot == 0 and cap > FIRST_CHUNK + 128:
        rest = cap - FIRST_CHUNK
        n = max(1, math.ceil(rest / MAX_CHUNK))
        while rest % n or (rest // n) % 2:
            n += 1
        sizes = [FIRST_CHUNK] + [rest // n] * n
    else:
        n = max(1, math.ceil(cap / MAX_CHUNK))
        while cap % n or (cap // n) % 2:
            n += 1
        sizes = [cap // n] * n
    if slot == N_SLOTS - 1 and sizes[-1] >= 384:
        sizes[-1:] = [sizes[-1] - 128, 128]
    return sizes


def _round_cap(cap: int) -> int:
    """Round capacity up so it splits into equal, even chunks <= 512."""
    cap = max(cap, 256)
    n = max(1, math.ceil(cap / MAX_CHUNK))
    return 2 * n * math.ceil(cap / (2 * n))


def _plan_packing(counts: np.ndarray):
    """Split the largest expert groups in half until there are 24 pieces,
    sort descending, deal 8 pieces per slot group; caps = per-group max.
    Returns (caps, assign): assign[s][c]=(expert,start,len)."""
    frags = [(int(e), 0, int(c)) for e, c in enumerate(counts) if c > 0]
    target = N_CORES * N_SLOTS
    assert len(frags) <= target, (
        f"{len(frags)} non-empty experts exceed {target} slots"
    )
    while len(frags) < target:
        frags.sort(key=lambda f: -f[2])
        e, st, ln = frags[0]
        if ln < 2:
            frags.append((e, st, 0))
            continue
        h1 = ln // 2
        frags[0] = (e, st, ln - h1)
        frags.append((e, st + (ln - h1), h1))
    frags.sort(key=lambda f: -f[2])
    caps, assign = [], []
    for s in range(N_SLOTS):
        group = frags[s * N_CORES : (s + 1) * N_CORES]
        caps.append(_round_cap(max(f[2] for f in group)))
        assign.append(group)
    return caps, assign


def _blob_layout(caps, chunk_lists):
    """Free-dim (fp16 element) offsets inside each slot's blob.
    Slot 0 blob: [w1_d0 | x_chunk0] (gates the first matmul with ONE sem).
    Slots 1+:   [w1_d0..d5 | x_chunk0.. | w2].
    Returns per-slot dicts with offsets and total width."""
    lay = []
    for s in range(N_SLOTS):
        d = {}
        off = 0
        if s == 0:
            d["w1"] = [0]  # only d0 in the blob
            off = H
            d["x"] = [off]
            off += DT * chunk_lists[s][0]
        else:
            d["w1"] = [off + dt * H for dt in range(DT)]
            off += DT * H
            d["x"] = []
            for sz in chunk_lists[s]:
                d["x"].append(off)
                off += DT * sz
            d["w2"] = off
            off += HT * W2M
        d["width"] = off
        lay.append(d)
    return lay


_PROGRAM_CACHE: dict = {}


def _build_program(key):
    if key in _PROGRAM_CACHE:
        return _PROGRAM_CACHE[key]
    caps, b1z, b2z = key

    R = sum(caps)
    chunk_lists = [_chunk_list(caps[s], s) for s in range(N_SLOTS)]
    lay = _blob_layout(caps, chunk_lists)

    nc = bacc.Bacc(
        "TRN2", target_bir_lowering=False, debug=False, num_devices=N_CORES
    )
    blob = [
        nc.dram_tensor(f"blob{s}", [128, lay[s]["width"]], MM_DT,
                       kind="ExternalInput")
        for s in range(N_SLOTS)
    ]
    # slot-0 extras (fine-grained so dt-major chunk 0 streams per-slab)
    w1r = nc.dram_tensor("w1r", [DT - 1, 128, H], MM_DT, kind="ExternalInput")
    x0r_w = DT * sum(chunk_lists[0][1:])
    x0r = nc.dram_tensor("x0r", [128, max(x0r_w, 1)], MM_DT,
                         kind="ExternalInput")
    w20 = nc.dram_tensor("w20", [128, HT * W2M], MM_DT, kind="ExternalInput")
    if L2_PACK:
        sel = nc.dram_tensor("sel", [128, C], MM_DT, kind="ExternalInput")
    if not b1z:
        b1 = nc.dram_tensor("b1", [N_SLOTS, 128, HT], F32,
                            kind="ExternalInput")
    if not b2z:
        b2 = nc.dram_tensor("b2", [N_SLOTS, C, 1], F32, kind="ExternalInput")
    y = nc.dram_tensor("y", [C, R], F32, kind="ExternalOutput")

    with TileContext(nc) as tc:
        with (
            tc.tile_pool(name="wpool", bufs=1) as wpool,
            tc.tile_pool(name="xpool", bufs=1) as xpool,
            tc.tile_pool(name="hpool", bufs=4) as hpool,
            tc.tile_pool(name="ypool", bufs=1) as ypool,
            tc.tile_pool(name="pspool", bufs=6, space="PSUM") as pspool,
            tc.tile_pool(name="pypool", bufs=2, space="PSUM") as pypool,
        ):
            # y staging lives on partitions 64-65: the selector matmul runs
            # as a column-group at offset 64, concurrent with the L2 group
            # block's second round (which only occupies groups 0 and 32) --
            # the selector disappears from the PE critical path entirely.
            y_sb = ypool.tile([66, R], F32, name="y_sb")

            # PE warm-up: fp32 LOW_HIGH pairs are slow at ramp speed,
            # bridging the ~4.5us until the first chunk's data lands
            # without a PE gap (a gap re-triggers the HAM 50% throttle).
            # memset the warmup operands from the Vector queue, which exits
            # the preamble ~0.4us before GpSimd -> the clock ramp (HAM full
            # grant comes ~6us after the first PE activity) starts earlier
            wu_w = ypool.tile([128, 128], F32, name="wu_w")
            wu_x = ypool.tile([128, 256], F32, name="wu_x")
            nc.vector.memset(wu_w[:, :], 0.0)
            nc.vector.memset(wu_x[:, :], 0.0)
            wu_ps = pspool.tile([128, 512], F32, name="wu_ps", tag="psh")
            for _ in range(5):
                nc.tensor.matmul(
                    wu_ps[:, :256], wu_w, wu_x, start=True, stop=True
                )
            # dummy relu up front so the 1.3us ACT_TABLE_LOAD happens during
            # the DMA fill instead of right before chunk 0's first relu
            act_warm = ypool.tile([1, 4], F32, name="act_warm")
            nc.scalar.activation(
                act_warm, wu_w[0:1, 0:4], mybir.ActivationFunctionType.Relu
            )

            # ---- tiles ----
            blob_t = [
                xpool.tile([128, lay[s]["width"]], MM_DT, name=f"blob{s}",
                           tag=f"blob{s}")
                for s in range(N_SLOTS)
            ]
            w1r_t = [
                wpool.tile([128, H], MM_DT, name=f"w1r_{dt}", tag=f"w1r_{dt}")
                for dt in range(DT - 1)
            ]
            x0r_t = []
            o = 0
            for ci in range(1, len(chunk_lists[0])):
                sz = chunk_lists[0][ci]
                x0r_t.append((o, xpool.tile([128, DT, sz], MM_DT,
                                            name=f"x0r_{ci}",
                                            tag=f"x0r_{ci}")))
                o += DT * sz
            w20_t = wpool.tile([128, HT * W2M], MM_DT, name="w20", tag="w20")
            if not b1z:
                b1_t = [
                    wpool.tile([128, HT], F32, name=f"b1_{s}", tag=f"b1_{s}")
                    for s in range(N_SLOTS)
                ]
            if not b2z:
                b2_t = [
                    wpool.tile([C, 1], F32, name=f"b2_{s}", tag=f"b2_{s}")
                    for s in range(N_SLOTS)
                ]
            if L2_PACK:
                sel_sb = ypool.tile([128, C], MM_DT, name="sel_sb")

            # ---- AP accessors ----
            def w1ap(s, dt, ht):
                if s == 0:
                    base = w1r_t[dt - 1] if dt >= 1 else blob_t[0]
                    o = (0 if dt >= 1 else lay[0]["w1"][0]) + ht * 128
                    return base[:, o : o + 128]
                o = lay[s]["w1"][dt] + ht * 128
                return blob_t[s][:, o : o + 128]

            def xap(s, ci, dt):
                sz = chunk_lists[s][ci]
                if s == 0:
                    if ci == 0:
                        o = lay[0]["x"][0] + dt * sz
                        return blob_t[0][:, o : o + sz]
                    _, t = x0r_t[ci - 1]
                    return t[:, dt, :]
                o = lay[s]["x"][ci] + dt * sz
                return blob_t[s][:, o : o + sz]

            def w2ap(s, ht, m):
                if s == 0:
                    return w20_t[:, ht * W2M : ht * W2M + m]
                o = lay[s]["w2"] + ht * W2M
                return blob_t[s][:, o : o + m]

            # ---- DMA issue schedule (single sync HWDGE queue) ----
            # blob0 first: ONE semaphore gates the first matmul (w1_d0 +
            # x_chunk0 together), then the remaining slot-0 w1 slabs in
            # dt-consumption order, then x for slot-0's later chunks.
            nc.sync.dma_start(out=blob_t[0], in_=blob[0][:, :])
            for dt in range(1, DT):
                nc.sync.dma_start(out=w1r_t[dt - 1], in_=w1r[dt - 1])
            first = True
            for (o, t), sz in zip(x0r_t, chunk_lists[0][1:]):
                if first:
                    # split in dt-halves: chunk 1 runs dt-major and starts
                    # on the first half, ~0.8us before the full transfer
                    nc.sync.dma_start(
                        out=t[:, 0:3, :], in_=x0r[:, o : o + 3 * sz]
                    )
                    nc.sync.dma_start(
                        out=t[:, 3:6, :],
                        in_=x0r[:, o + 3 * sz : o + 6 * sz],
                    )
                    first = False
                else:
                    nc.sync.dma_start(out=t, in_=x0r[:, o : o + DT * sz])
            nc.sync.dma_start(out=w20_t, in_=w20[:, :])
            if L2_PACK:
                nc.sync.dma_start(out=sel_sb[:, :], in_=sel[:, :])
            for s in range(1, N_SLOTS):
                nc.sync.dma_start(out=blob_t[s], in_=blob[s][:, :])
            if not b1z:
                for s in range(N_SLOTS):
                    nc.sync.dma_start(out=b1_t[s], in_=b1[s])
            if not b2z:
                for s in range(N_SLOTS):
                    nc.sync.dma_start(out=b2_t[s], in_=b2[s])

            # ---- compute: software-pipelined across chunks ----
            slot_offs = np.cumsum([0] + list(caps[:-1])).tolist()
            chunks = []  # (slot, ci, size, y_off, last_of_slot)
            for s in range(N_SLOTS):
                co = 0
                for ci, size in enumerate(chunk_lists[s]):
                    chunks.append(
                        (s, ci, size, slot_offs[s] + co,
                         ci == len(chunk_lists[s]) - 1)
                    )
                    co += size
            state = {}

            def emit_l1(k, dt_major):
                s, ci, size, _, _ = chunks[k]
                h_sb = hpool.tile([128, HT, size], MM_DT, name="h_sb",
                                  tag="h")
                def do_relu(ht, ps):
                    if b1z:
                        nc.scalar.activation(
                            h_sb[:, ht, :], ps,
                            mybir.ActivationFunctionType.Relu,
                        )
                    else:
                        nc.scalar.activation(
                            h_sb[:, ht, :], ps,
                            mybir.ActivationFunctionType.Relu,
                            bias=b1_t[s][:, ht : ht + 1],
                        )
                if dt_major:
                    ps6 = [
                        pspool.tile([128, size], F32, name=f"ps_h{ht}",
                                    tag="psh")
                        for ht in range(HT)
                    ]
                    for dt in range(DT):
                        for ht in range(HT):
                            nc.tensor.matmul(
                                ps6[ht],
                                w1ap(s, dt, ht),
                                xap(s, ci, dt),
                                start=(dt == 0),
                                stop=(dt == DT - 1),
                            )
                    for ht in range(HT):
                        do_relu(ht, ps6[ht])
                else:
                    for ht in range(HT):
                        ps_h = pspool.tile([128, size], F32, name="ps_h",
                                           tag="psh")
                        for dt in range(DT):
                            nc.tensor.matmul(
                                ps_h,
                                w1ap(s, dt, ht),
                                xap(s, ci, dt),
                                start=(dt == 0),
                                stop=(dt == DT - 1),
                            )
                        do_relu(ht, ps_h)
                state[k] = {"h_sb": h_sb}

            def emit_grp(k):
                s, ci, size, _, _ = chunks[k]
                st = state[k]
                h_sb = st["h_sb"]
                # last chunk: plain serial L2 — skipping the copy+selector
                # shortens the exit chain after the final L1 matmul
                if L2_PACK and k != len(chunks) - 1:
                    ps_y4 = pypool.tile([128, size], F32, name="ps_y4",
                                        tag="psy")
                    for ht in range(HT):
                        g = ht % 4
                        nc.tensor.matmul(
                            ps_y4[32 * g : 32 * g + L2_M, :],
                            w2ap(s, ht, L2_M),
                            h_sb[:, ht, :],
                            start=(ht < 4),
                            stop=(ht >= 4 or g >= HT - 4),
                            tile_position=(0, 32 * g),
                        )
                    y4_sb = hpool.tile([128, size], MM_DT, name="y4_sb",
                                       tag="y4")
                    if k == len(chunks) - 2:
                        # last packed chunk: copy on Vector so its selector
                        # is not gated behind the tail chunk's relus on
                        # Scalar (all-vector copies measured slower mid-run)
                        nc.vector.tensor_scalar_add(y4_sb, ps_y4, 0.0)
                    else:
                        nc.scalar.activation(
                            y4_sb, ps_y4, mybir.ActivationFunctionType.Copy
                        )
                    st["y4_sb"] = y4_sb
                else:
                    # pspool, not pypool: at the tail pypool's buffers are
                    # held by the previous chunk's copy/selector chain.
                    # Output on partitions 64:66 (column group 64) so it can
                    # overlap the previous chunk's group block.
                    ps_f = pspool.tile([128, size], F32, name="ps_y",
                                       tag="psh")
                    ps_y = ps_f[64:66, :]
                    for ht in range(HT):
                        nc.tensor.matmul(
                            ps_y,
                            w2ap(s, ht, C),
                            h_sb[:, ht, :],
                            start=(ht == 0),
                            stop=(ht == HT - 1),
                            tile_position=(0, 64),
                        )
                    st["ps_y"] = ps_y

            def emit_sel_add(k):
                s, ci, size, y_off, last = chunks[k]
                st = state[k]
                if "ps_y" in st:
                    ps_y = st["ps_y"]
                else:
                    ps_f = pypool.tile([128, size], F32, name="ps_y",
                                       tag="psy")
                    ps_y = ps_f[64:66, :]
                    nc.tensor.matmul(ps_y, sel_sb, st["y4_sb"], start=True,
                                     stop=True, tile_position=(0, 64))
                bias = 0.0 if b2z else b2_t[s][:, :]
                nc.vector.tensor_scalar_add(
                    y_sb[64:66, y_off : y_off + size], ps_y, bias
                )
                if last:
                    nc.sync.dma_start(
                        out=y[:, slot_offs[s] : slot_offs[s] + caps[s]],
                        in_=y_sb[64:66,
                                 slot_offs[s] : slot_offs[s] + caps[s]],
                    )
                del state[k]

            nchunks = len(chunks)
            for k in range(nchunks):
                emit_l1(k, dt_major=(k <= 1))
                if k >= 1:
                    emit_grp(k - 1)
                if k >= 2:
                    emit_sel_add(k - 2)
            emit_grp(nchunks - 1)
            if nchunks >= 2:
                emit_sel_add(nchunks - 2)
            emit_sel_add(nchunks - 1)
    nc.compile()
    _PROGRAM_CACHE[key] = nc
    return nc


def kernel(embeddings, component_idx, W1, b1, W2, b2):
    embeddings = np.ascontiguousarray(np.asarray(embeddings, dtype=np.float32))
    ci = np.asarray(component_idx).astype(np.int64, copy=False)
    W1 = np.asarray(W1, dtype=np.float32)
    b1 = np.asarray(b1, dtype=np.float32)
    W2 = np.asarray(W2, dtype=np.float32)
    b2 = np.asarray(b2, dtype=np.float32)

    N = embeddings.shape[0]
    E = W1.shape[0]

    counts = np.bincount(ci, minlength=E)
    order = np.argsort(ci, kind="stable")
    group_start = np.zeros(E, dtype=np.int64)
    group_start[1:] = np.cumsum(counts)[:-1]
    x_sorted = embeddings[order]  # [N, D] grouped by expert

    caps, assign = _plan_packing(counts)
    R = sum(caps)
    offs = np.cumsum([0] + caps[:-1]).tolist()
    chunk_lists = [_chunk_list(caps[s], s) for s in range(N_SLOTS)]
    lay = _blob_layout(caps, chunk_lists)

    b1z = not np.any(b1)
    b2z = not np.any(b2)
    key = (tuple(caps), b1z, b2z)
    nc = _build_program(key)
    global _LAST_NC
    _LAST_NC = nc

    # host-side packing
    w1_packed = W1.reshape(E, DT, 128, H).astype(MM_NP)  # [e, dt, din, h]
    if not b1z:
        b1_packed = np.ascontiguousarray(
            b1.reshape(E, HT, 128).transpose(0, 2, 1)
        )
    w2_packed = np.zeros((E, 128, HT * W2M), dtype=MM_NP)
    w2v = W2.reshape(E, HT, 128, C).transpose(0, 2, 1, 3)  # [e,128,ht,C]
    for ht in range(HT):
        w2_packed[:, :, ht * W2M : ht * W2M + C] = w2v[:, :, ht]
    if not b2z:
        b2_packed = b2.reshape(E, C, 1)

    def xblock(Xc, tok, sz):
        blk = Xc[tok : tok + sz].T.astype(MM_NP)  # [768, sz]
        return blk.reshape(DT, 128, sz).transpose(1, 0, 2).reshape(128, -1)

    in_maps = []
    for c in range(N_CORES):
        im = {}
        Xc = np.zeros((R, D), dtype=np.float32)
        experts = []
        for s in range(N_SLOTS):
            e, st, ln = assign[s][c]
            experts.append(e)
            beg = group_start[e] + st
            Xc[offs[s] : offs[s] + ln] = x_sorted[beg : beg + ln]
        # blobs
        tok = 0
        for s in range(N_SLOTS):
            e = experts[s]
            bl = np.empty((128, lay[s]["width"]), dtype=MM_NP)
            if s == 0:
                bl[:, 0:H] = w1_packed[e, 0]
                sz0 = chunk_lists[0][0]
                bl[:, H : H + DT * sz0] = xblock(Xc, tok, sz0)
                tok += sz0
                # slot-0 extras
                im["w1r"] = w1_packed[e, 1:]
                parts = []
                for cix in range(1, len(chunk_lists[0])):
                    sz = chunk_lists[0][cix]
                    parts.append(xblock(Xc, tok, sz))
                    tok += sz
                im["x0r"] = (
                    np.concatenate(parts, axis=1)
                    if parts
                    else np.zeros((128, 1), dtype=MM_NP)
                )
                im["w20"] = w2_packed[e]
                if L2_PACK:
                    sel_np = np.zeros((128, C), dtype=MM_NP)
                    for g in range(4):
                        for cc in range(C):
                            sel_np[32 * g + cc, cc] = 1
                    im["sel"] = sel_np
            else:
                for dt in range(DT):
                    o = lay[s]["w1"][dt]
                    bl[:, o : o + H] = w1_packed[e, dt]
                for cix, sz in enumerate(chunk_lists[s]):
                    o = lay[s]["x"][cix]
                    bl[:, o : o + DT * sz] = xblock(Xc, tok, sz)
                    tok += sz
                o = lay[s]["w2"]
                bl[:, o : o + HT * W2M] = w2_packed[e]
            im[f"blob{s}"] = bl
        if not b1z:
            im["b1"] = np.stack([b1_packed[e] for e in experts])
        if not b2z:
            im["b2"] = np.stack([b2_packed[e] for e in experts])
        in_maps.append(im)

    global _LAST_IN_MAPS
    _LAST_IN_MAPS = in_maps
    res = run_bass_kernel_spmd(nc, in_maps, list(range(N_CORES)))

    out = np.empty((N, C), dtype=np.float32)
    for c in range(N_CORES):
        yc = res.results[c]["y"]  # [C, R]
        for s in range(N_SLOTS):
            e, st, ln = assign[s][c]
            if ln == 0:
                continue
            beg = group_start[e] + st
            tokens = order[beg : beg + ln]
            out[tokens] = yc[:, offs[s] : offs[s] + ln].T
    return out
